# revision 2
# baseline (speedup 1.0000x reference)
"""CrossAttentionS2T (attn_all_frame=True) — fp8 Bass/Tile kernel, 8 trn2 cores.

Data-parallel over batch B=8 (one batch element per core). Per core:

  Host folds positional embeddings into the activations, transposes to
  [feature, token], scales weights by 8, and casts everything to fp8 e4m3
  in DoubleRow layout [128, 6, tokens] (6 contraction sub-tiles of 128).

  Q/K proj  : fp8 DoubleRow matmuls (0.5 cyc/row), transposed out [feat, tok]
  V proj    : fp8 DoubleRow, natural out [ktok, feat] (+1/16 ones column
              per head -> softmax denominator/16 rides the attnV matmul)
  scores    : fp8 matmuls, out [ktok, qtok]; two 112-k-blocks share one
              2-bank PSUM tile so ONE activation exps 784 columns
  exp       : split between ACT (true Exp, scale=1/512) and DVE
              (Schraudolph: i8 = round(A*x+B) bit-cast as e4m3 ~= exp)
  attnV     : fp8 DoubleRow over k-block pairs; out [65, qt]: 64 o rows +
              denominator row
  normalize : reciprocal_approx_fast on the denom row, PE broadcast via a
              block-diagonal ones matmul, one DVE multiply per head
  out proj  : fp8 DoubleRow, natural out [qtok, feat]; bias (Wp@v_bias+pb,
              scaled) injected via an f32r rank-1 matmul into PSUM; bf16 out

  Scale bookkeeping: weights x8 -> scores x64 (folded into exp scale
  1/512 = 0.125/64); v x8, ones col 1/16 -> oT = 128 x o_attn; Wp x8 ->
  device output = 1024 x reference; host divides by 1024.
"""

import math
import os
from contextlib import ExitStack

import numpy as np

import concourse.bass as bass
import concourse.mybir as mybir
import concourse.tile as tile
from concourse.bass import ds, ts

F32 = mybir.dt.float32
F32R = mybir.dt.float32r
F8 = mybir.dt.float8e4
I8 = mybir.dt.int8
I32 = mybir.dt.int32
BF16 = mybir.dt.bfloat16
AF = mybir.ActivationFunctionType
DR = mybir.MatmulPerfMode.DoubleRow

# problem dims (hardcoded per contract)
B, SPEC, T = 8, 4, 8
AP_, VP, DIM = 196, 196, 768
NH, HD = 12, 64
SCALE = HD ** -0.5          # 0.125
NQ = VP * T                 # 1568 q tokens per batch
NK = AP_ * SPEC             # 784 kv tokens per batch
DC = DIM // 128             # 6 contraction chunks
QT, NQT = 392, 4            # q-token tile
KB, NJ = 112, 7             # k-token block
NCORES = 8

WS = 8.0                    # weight scale (q,k,v,p all x8)
OUT_SCALE = 1024.0          # 8(v) * 16(ones=1/16) * 8(wp)
ONES_VAL = 1.0 / 16.0
EXP_SCALE = SCALE / (WS * WS)          # 1/512
A8 = (8.0 / math.log(2.0)) * EXP_SCALE  # schraudolph mult
B8 = 55.5                                # schraudolph bias (e4m3 bits)

# q-block coverage per qt group for the interleaved out-projection
QB_OF_QT = [range(0, 3), range(3, 6), range(6, 9), range(9, 13)]

# debug toggles (bisection of sim/hw divergence)
USE_SCHRAUDOLPH = os.environ.get("K2_SCHRAUDOLPH", "1") == "1"
USE_DR_ATTNV = os.environ.get("K2_DR_ATTNV", "1") == "1"
USE_DR_PROJ = os.environ.get("K2_DR_PROJ", "1") == "1"
USE_PAIREXP = os.environ.get("K2_PAIREXP", "1") == "1"
USE_FASTRECIP = os.environ.get("K2_FASTRECIP", "1") == "1"
USE_MMBIAS = os.environ.get("K2_MMBIAS", "1") == "1"
USE_POOLBCAST = os.environ.get("K2_POOLBCAST", "1") == "1"
USE_DR_SCORES = os.environ.get("K2_DR_SCORES", "1") == "1"


def _r(ap):
    return ap.bitcast(F32R)


def _emit(ctx, tc, outs, ins):
    nc = tc.nc
    (txdr, sxdr, wqdr, wkdr, wvdr, wpdr, qb2, kb2) = ins
    (out_d,) = outs
    alp = nc.allow_low_precision(reason="fp8 kernel, tolerance 2e-2")
    ctx.enter_context(alp)

    const = ctx.enter_context(tc.tile_pool(name="const", bufs=1))
    qb_t = const.tile([128, DC], F32)
    kb_t = const.tile([128, DC], F32)
    nc.sync.dma_start(qb_t[:], qb2[:])
    nc.sync.dma_start(kb_t[:], kb2[:])
    ones1_tmp = const.tile([1, 128], F32)
    nc.gpsimd.memset(ones1_tmp[:], 1.0)
    ones1 = const.tile([1, 128], F32)
    nc.vector.tensor_copy(_r(ones1[:]), ones1_tmp[:])
    ones1b = const.tile([1, 128], BF16)
    nc.gpsimd.memset(ones1b[:], 1.0)

    # persistent weights + activations
    pers = ctx.enter_context(tc.tile_pool(name="pers", bufs=1))
    wq_t = pers.tile([128, DC, DIM], F8)
    wk_t = pers.tile([128, DC, DIM], F8)
    wv_t = pers.tile([128, DC, DIM], F8)
    wp_t = pers.tile([128, DC, DIM], F8)
    tx_t = pers.tile([128, DC, NQ], F8)
    sx_t = pers.tile([128, DC, NK], F8)
    for dst, dsrc in ((wk_t, wkdr), (sx_t, sxdr), (wv_t, wvdr),
                      (wq_t, wqdr), (tx_t, txdr), (wp_t, wpdr)):
        nc.sync.dma_start(dst[:], dsrc[:])

    qTt = [pers.tile([128, NQ], F8, name=f"qT{c}", tag=f"qT{c}") for c in range(DC)]
    kTt = [pers.tile([128, NK], F8, name=f"kT{c}", tag=f"kT{c}") for c in range(DC)]

    # v in natural layout [ktok, head-slot] with a 1/16 column per head;
    # j-block pairs share a tile (DoubleRow k-tile dim). Head slots padded
    # to 68 so the DoubleRow k-tile stride (NH*68) is 16B-aligned.
    HS = 68
    HS2 = 80               # head-major slot: 2 k-tiles x 80B, 16B-aligned
    KBF, KTL = 128, 16     # six full 128-token k-blocks + a 16-token tail
    v_st = [pers.tile([128, NH, 2, HS2], F8, name=f"v{p}", tag=f"v{p}")
            for p in range(3)]
    v_s6 = pers.tile([128, NH, HS2], F8)
    oT = pers.tile([128, DC, NQ], F8)

    for p in range(3):
        nc.vector.memset(v_st[p][0:KBF, :, :, ds(HD, 1)], ONES_VAL)
        nc.vector.memset(v_st[p][0:KBF, :, :, ds(HD + 1, HS2 - HD - 1)], 0.0)
    nc.vector.memset(v_s6[0:KTL, :, ds(HD, 1)], ONES_VAL)
    nc.vector.memset(v_s6[0:KTL, :, ds(HD + 1, HS2 - HD - 1)], 0.0)

    # PSUM: pp (proj + rb + out-proj) 2 banks, spair 2, ssing 1, o 3 = 8
    psP = ctx.enter_context(tc.tile_pool(name="psP", bufs=2, space="PSUM"))
    psS = ctx.enter_context(tc.tile_pool(name="psS", bufs=1, space="PSUM"))
    psO = ctx.enter_context(tc.tile_pool(name="psO", bufs=2, space="PSUM"))

    work = ctx.enter_context(tc.tile_pool(name="work", bufs=1))

    # ---- K projection: kT[f][:, tok] ----
    for f in range(DC):
        for half in range(2):
            ps = psP.tile([128, QT], F32, name="ps_k", tag="pp")
            if USE_DR_PROJ:
                for cp in range(3):
                    nc.tensor.matmul(
                        ps[:], wk_t[:, ds(2 * cp, 2), ts(f, 128)],
                        sx_t[:, ds(2 * cp, 2), ts(half, QT)],
                        start=(cp == 0), stop=(cp == 2), perf_mode=DR)
            else:
                for c in range(DC):
                    nc.tensor.matmul(
                        ps[:], wk_t[:, ds(c, 1), ts(f, 128)],
                        sx_t[:, ds(c, 1), ts(half, QT)],
                        start=(c == 0), stop=(c == DC - 1))
            nc.vector.tensor_scalar_add(kTt[f][:, ts(half, QT)], ps[:],
                                        kb_t[:, ds(f, 1)])

    # ---- V projection: natural [ktok, feat] ----
    for j in range(NJ):
        kb = KBF if j < 6 else KTL
        for w in range(2):
            ps = psP.tile([128, 6, HD], F32, name="ps_v", tag="pp")
            for cp in range(3):
                nc.tensor.matmul(
                    ps[0:kb, :, :], sx_t[:, ds(2 * cp, 2), ds(j * KBF, kb)],
                    wv_t[:, ds(2 * cp, 2), ts(w, 384)],
                    start=(cp == 0), stop=(cp == 2), perf_mode=DR)
            if j < 6:
                dst = v_st[j // 2][0:kb, ds(6 * w, 6), ds(j % 2, 1), 0:HD]
            else:
                dst = v_s6[0:kb, ds(6 * w, 6), 0:HD]
            nc.vector.tensor_copy(dst, ps[0:kb, :, :])

    # ---- Q projection: qT[f][:, tok] ----
    for qt in range(NQT):
        for f in range(DC):
            ps = psP.tile([128, QT], F32, name="ps_q", tag="pp")
            if USE_DR_PROJ:
                for cp in range(3):
                    nc.tensor.matmul(
                        ps[:], wq_t[:, ds(2 * cp, 2), ts(f, 128)],
                        tx_t[:, ds(2 * cp, 2), ts(qt, QT)],
                        start=(cp == 0), stop=(cp == 2), perf_mode=DR)
            else:
                for c in range(DC):
                    nc.tensor.matmul(
                        ps[:], wq_t[:, ds(c, 1), ts(f, 128)],
                        tx_t[:, ds(c, 1), ts(qt, QT)],
                        start=(c == 0), stop=(c == DC - 1))
            nc.scalar.activation(qTt[f][:, ts(qt, QT)], ps[:], AF.Identity,
                                 bias=qb_t[:, ds(f, 1)])

    # ---- attention + interleaved out-projection ----
    def out_proj(qb):
        qw = min(128, NQ - qb * 128)
        onat = work.tile([128, DIM], BF16, name="onat", tag="onat", bufs=2)
        for w in range(2):
            po = psP.tile([128, 384], F32, name="po", tag="pp")
            if USE_DR_PROJ:
                for cp in range(3):
                    nc.tensor.matmul(
                        po[0:qw, :],
                        oT[:, ds(2 * cp, 2), ds(qb * 128, qw)],
                        wp_t[:, ds(2 * cp, 2), ds(w * 384, 384)],
                        start=(cp == 0), stop=(cp == 2), perf_mode=DR)
            else:
                for c in range(DC):
                    nc.tensor.matmul(
                        po[0:qw, :],
                        oT[:, ds(c, 1), ds(qb * 128, qw)],
                        wp_t[:, ds(c, 1), ds(w * 384, 384)],
                        start=(c == 0), stop=(c == DC - 1))
            nc.scalar.activation(onat[0:qw, ds(w * 384, 384)], po[0:qw, :],
                                 AF.Copy)
        nc.sync.dma_start(out_d[ds(qb * 128, qw), :], onat[0:qw, :])

    pending_qbs = []
    for qt in range(NQT):
        for hp in range(6):
            if hp % 2 == 1 and pending_qbs:
                out_proj(pending_qbs.pop(0))
            for hh in range(2):
                h = 2 * hp + hh
                off = hh * HD
                # 16-token tail block first (its exp gates the attnV start)
                s1t = psS.tile([128, 2, 512], F32, name="s1t", tag="spair",
                               bufs=2)
                s1 = s1t[:, 0, 0:QT]
                nc.tensor.matmul(s1[0:KTL, :],
                                 kTt[hp][ds(off, HD), ds(6 * KBF, KTL)],
                                 qTt[hp][ds(off, HD), ts(qt, QT)],
                                 start=True, stop=True)
                p1 = work.tile([128, QT], F8, name="p1", tag="p1", bufs=2)
                if USE_SCHRAUDOLPH:
                    nc.vector.tensor_scalar(
                        p1[0:KTL, :].bitcast(I8), s1[0:KTL, :],
                        A8, B8, mybir.AluOpType.mult, mybir.AluOpType.add)
                else:
                    nc.scalar.activation(p1[0:KTL, :], s1[0:KTL, :], AF.Exp,
                                         scale=EXP_SCALE)
                # scores in j-pairs -> one exp per 784 columns
                pjs = []
                for jp in range(3):
                    pj = work.tile([128, 2, QT], F8, name="pj", tag=f"pj{jp}",
                                   bufs=2)
                    sp = psS.tile([128, 2, 512], F32, name="sp",
                                  tag="spair", bufs=2)
                    for jj in range(2):
                        j = 2 * jp + jj
                        nc.tensor.matmul(
                            sp[:, ds(jj, 1), 0:QT],
                            kTt[hp][ds(off, HD), ts(j, KBF)],
                            qTt[hp][ds(off, HD), ts(qt, QT)],
                            start=True, stop=True)
                    if jp != 1 or not USE_SCHRAUDOLPH:
                        nc.scalar.activation(pj[:, :, :],
                                             sp[:, :, 0:QT], AF.Exp,
                                             scale=EXP_SCALE)
                    else:
                        nc.vector.tensor_scalar(
                            pj[:, :, :].bitcast(I8), sp[:, :, 0:QT],
                            A8, B8, mybir.AluOpType.mult,
                            mybir.AluOpType.add)
                    pjs.append(pj)
                # attn @ V (+ denominator row); tail block first
                op = psO.tile([128, QT], F32, name="op", tag="o")
                nc.tensor.matmul(op[0:HS2, :],
                                 v_s6[0:KTL, ds(h, 1), :],
                                 p1[0:KTL, :], start=True, stop=False)
                for jp in range(3):
                    nc.tensor.matmul(
                        op[0:HS2, :],
                        v_st[jp][:, h, :, :],
                        pjs[jp][:, :, :],
                        start=False, stop=(jp == 2), perf_mode=DR)
                # magic-number reciprocal: r_bits = K - d_bits, done as
                # (d_bits - K')*(-1); +0x8000 pre-rounds the bf16 truncation.
                rm = work.tile([1, QT, 2], BF16, name="rm", tag="rm",
                               bufs=3)
                nc.vector.tensor_scalar(
                    rm[:].bitcast(I32), op[ds(HD, 1), :].bitcast(I32),
                    0x7EF2F800 + 0x8000, -1,
                    mybir.AluOpType.subtract, mybir.AluOpType.mult)
                if USE_POOLBCAST:
                    rbb = work.tile([HD, QT], BF16, name="rbb", tag="rbb",
                                    bufs=3)
                    nc.gpsimd.partition_broadcast(rbb[:], rm[0:1, :, ds(1, 1)])
                    nc.vector.tensor_mul(
                        oT[ds(off, HD), ds(hp, 1), ts(qt, QT)],
                        op[0:HD, :], rbb[:])
                else:
                    rb = psP.tile([128, QT], F32, name="rb", tag="pp")
                    nc.tensor.matmul(rb[0:HD, :], ones1b[0:1, 0:HD],
                                     rm[0:1, :, ds(1, 1)],
                                     start=True, stop=True)
                    rbs = work.tile([128, QT], F32, name="rbs", tag="rbs",
                                    bufs=3)
                    nc.scalar.activation(rbs[0:HD, :], rb[0:HD, :], AF.Copy)
                    nc.vector.tensor_mul(
                        oT[ds(off, HD), ds(hp, 1), ts(qt, QT)],
                        op[0:HD, :], rbs[0:HD, :])

        # q-blocks fully covered by this qt group become eligible;
        # they are emitted interleaved into the NEXT qt's head groups
        pending_qbs.extend(QB_OF_QT[qt])
    for qb in pending_qbs:
        out_proj(qb)


def build_program():
    from concourse import bacc
    from concourse.compiler_utils import get_compiler_flags, set_compiler_flags
    flags = [f.replace("--enable-ldw-opt=false", "--enable-ldw-opt=true")
             for f in get_compiler_flags()]
    set_compiler_flags(flags)
    nc = bacc.Bacc("TRN2", target_bir_lowering=False, debug=False,
                   num_devices=NCORES)
    mk = lambda name, shape, dt, out=False: nc.dram_tensor(
        name, shape, dt, kind="ExternalOutput" if out else "ExternalInput").ap()
    ins = [
        mk("txdr", [128, DC, NQ], F8), mk("sxdr", [128, DC, NK], F8),
        mk("wqdr", [128, DC, DIM], F8), mk("wkdr", [128, DC, DIM], F8),
        mk("wvdr", [128, DC, DIM], F8), mk("wpdr", [128, DC, DIM], F8),
        mk("qb2", [128, DC], F32), mk("kb2", [128, DC], F32),
    ]
    outs = [mk("out", [NQ, DIM], BF16, out=True)]
    with tile.TileContext(nc) as tc:
        with ExitStack() as ctx:
            _emit(ctx, tc, outs, ins)
    nc.compile()
    return nc


def host_prep(inputs):
    """Fold positions into activations, scale weights by 8, transpose to
    [feat, tok], pack in DoubleRow layout [128, 6, tok], cast fp8."""
    f32 = np.float32
    f8 = mybir.dt.np(F8)
    g = {k: np.asarray(v, dtype=f32) for k, v in inputs.items()}

    def drpack(matT):  # [DIM, N] f32 -> [128, 6, N] fp8
        n = matT.shape[1]
        return np.ascontiguousarray(
            matT.reshape(DC, 128, n).transpose(1, 0, 2)).astype(f8)

    wq = drpack(WS * g["Wq"].T)
    wk = drpack(WS * g["Wkv"][:DIM].T)
    wv = drpack(WS * g["Wkv"][DIM:].T)
    wp = drpack(WS * g["Wproj"].T)
    qb2 = np.ascontiguousarray((WS * g["q_bias"]).reshape(DC, 128).T)
    kb2 = np.ascontiguousarray((WS * g["kv_bias"][:DIM]).reshape(DC, 128).T)

    t_pat = g["t_x"][1:]                      # (VP, B*T, D)
    s_x = g["s_x"]                            # (AP, B*SPEC, D)
    tq = (t_pat.reshape(VP, B, T, DIM)
          + g["vmae_space_pos"][:, None, None, :]
          + g["vmae_temporal_pos"][None, None, :, :])
    sq = (s_x.reshape(AP_, B, SPEC, DIM)
          + g["clip_space_pos"][:, None, None, :]
          + g["clip_temporal_pos"][None, None, :, :])

    shared = dict(wqdr=wq, wkdr=wk, wvdr=wv, wpdr=wp,
                  qb2=qb2.astype(f32), kb2=kb2.astype(f32))
    in_maps = []
    for b in range(B):
        txT = tq[:, b].reshape(NQ, DIM).T      # (768, 1568)
        sxT = sq[:, b].reshape(NK, DIM).T      # (768, 784)
        in_maps.append(dict(txdr=drpack(txT), sxdr=drpack(sxT), **shared))
    return in_maps


def host_finish(results, inputs):
    g = lambda k: np.asarray(inputs[k], dtype=np.float32)
    out_bias = g("Wproj") @ g("kv_bias")[DIM:] + g("proj_bias")
    o = np.stack([np.asarray(results[b]["out"], dtype=np.float32)
                  for b in range(B)]) / OUT_SCALE + out_bias
    o = o.reshape(B, VP, T, DIM).transpose(1, 0, 2, 3).reshape(VP, B * T, DIM)
    return np.concatenate(
        [g("t_x")[0:1], o], axis=0)


_NC = None


def kernel(**inputs):
    global _NC
    from concourse.bass_utils import run_bass_kernel_spmd
    if _NC is None:
        _NC = build_program()
    in_maps = host_prep(inputs)
    res = run_bass_kernel_spmd(_NC, in_maps, list(range(NCORES)))
    return host_finish(res.results, inputs)
